# revision 17
# baseline (speedup 1.0000x reference)
"""Trainium2 Bass kernel for relational graph convolution:

    y = sum_r (A[r] @ x) @ W[r].T        A: [8, 4096, 4096] f32
                                         x: [4096, 64] f32, W: [8, 64, 64] f32

Strategy
--------
By associativity, y = sum_r A[r] @ v_r with v_r = x @ W[r].T, turning the
problem into one [4096, 4096] @ [4096, 64] matmul per relation. Relations are
sharded across the 8 NeuronCores (expert-style parallelism); each core returns
its partial y_r.T and the host sums and transposes.

The TensorE contracts over the partition dimension of both operands, so the
contraction index m (A's column index) must land on SBUF partitions. The host
therefore ships A[r].T (row-major) so device DMAs are plain contiguous slabs.

MODE="fp8dr" (default): the host ships e4m3(A[r].T - 0.5) — mean-subtracting
halves the quantized magnitude (and so the quantization error), and the
subtracted rank-1 term 0.5 * ones @ v_r is added back exactly on the host
(0.5 * (colsum x) @ W_r.T, identical for every output row). On device, v is
computed in exact fp32 and rounded to e4m3; phase 2 then runs
perf_mode=DoubleRow matmuls (both operands fp8): each instruction contracts
over 256 rows (2 interleaved 128-row blocks) at 2 elements/partition/cycle —
2x the bf16/f32r PE rate and 1/4 the f32 DMA traffic (16 MB/core). Measured
end-to-end rel error ~1.6e-2 (tolerance 2e-2), deterministic given the fixed
problem seed.

Per core (fp8dr):
  phase 1: v = x @ W_r.T via 32 exact-fp32 matmuls, rounded to e4m3 by the
           DVE copy into v_sb [128, 32, 64].
  phase 2: for each of 8 A-tiles (512 rows of A_r.T, 2 MB DMA): 2 DoubleRow
           pair-chunks x 8 PSUM banks of matmul(lhsT=v[128,2,64],
           rhs=A_t[128,2,512]) accumulating y_r.T [64, 4096] in fp32.
  phase 3: per-bank PSUM -> SBUF copies chase the final matmuls, then DMA
           y_r.T out; host sums partials, transposes, adds the rank-1 mean
           correction.

MODE="f32r" is the previous exact-ish variant (~1e-4 rel err, 64 MB/core,
DMA-bound); MODE="bf16" halves that traffic (~1.3e-3 rel err, 32 MB/core).
"""

import numpy as np

import concourse.tile as tile
from concourse import bacc, mybir
from concourse.bass_utils import run_bass_kernel_spmd

R, N, IN_F, OUT_F = 8, 4096, 64, 64
P = 128            # partition dim / contraction chunk
MC = N // P        # 32 contraction chunks
BANK = 512         # fp32 elems per PSUM bank
NB = N // BANK     # 8 output column blocks

F32 = mybir.dt.float32

MODE = "fp8dr"     # "fp8dr" (default), "f32r", or "bf16"
JC_FP8 = 4         # 128-row blocks per A DMA tile in fp8dr mode (even)
AT_BUFS_FP8 = 6    # A-tile prefetch depth in fp8dr mode

_NC_CACHE = {}

_A_DT = {
    "f32r": mybir.dt.float32r,
    "bf16": mybir.dt.bfloat16,
    "fp8dr": mybir.dt.float8e4,
}


def _build_nc(repeat=1, mode=None, jc=None, alt=True, at_bufs=None, variant="full"):
    """repeat>1 re-runs phase 2 (the steady-state A-streaming loop) that many
    times inside one NEFF — used only by the benchmark harness to amortize
    per-execute dispatch overhead; the graded kernel uses repeat=1.

    jc = 128-row chunks of A per DMA transfer (must be even for fp8dr);
    alt = alternate the two HWDGE rings (SP / ACT) between consecutive A-slab
    DMAs to hide the per-DMA completion gap.

    variant: "full" (the real kernel), "dma" (A-stream DMAs only — measures
    the DMA roofline; output is garbage), or "pe" (matmuls on stale SBUF, no
    A DMAs — measures the PE roofline; output is garbage). The roofline
    variants exist only for the benchmark harness."""
    mode = mode or MODE
    a_dt = _A_DT[mode]
    if jc is None:
        jc = {"f32r": 1, "bf16": 2, "fp8dr": JC_FP8}[mode]
    if at_bufs is None:
        at_bufs = {"f32r": {1: 4, 2: 3, 4: 2}[jc], "bf16": 4, "fp8dr": AT_BUFS_FP8}[
            mode
        ]
    dr = mode == "fp8dr"
    if dr:
        assert jc % 2 == 0
        perf_mode = mybir.MatmulPerfMode.DoubleRow

    nc = bacc.Bacc("TRN2", target_bir_lowering=False, debug=False, num_devices=R)

    if dr:
        # Host ships A_r.T pre-tiled as [c, p, j, n] so each partition's DMA
        # line per tile is one contiguous jc*4KB run (16 KB at jc=4) — same
        # descriptor efficiency as the f32 layout's 16 KB rows.
        at = nc.dram_tensor(
            "at", [MC // jc, P, jc, N], a_dt, kind="ExternalInput"
        ).ap()
    else:
        at = nc.dram_tensor("at", [N, N], a_dt, kind="ExternalInput").ap()
    ph1_dt = mybir.dt.float32r if dr else F32
    # bf16 partials halve phase-3 traffic; the host accumulates them in fp32.
    # The extra ~7e-4 relative error is far below the e4m3 quantization term.
    out_dt = mybir.dt.bfloat16 if dr else F32
    xt = nc.dram_tensor("xt", [IN_F, N], ph1_dt, kind="ExternalInput").ap()
    wt = nc.dram_tensor("wt", [IN_F, OUT_F], ph1_dt, kind="ExternalInput").ap()
    ytp = nc.dram_tensor("ytp", [OUT_F, N], out_dt, kind="ExternalOutput").ap()

    with tile.TileContext(nc) as tc:
        with (
            tc.tile_pool(name="const", bufs=1) as const_pool,
            tc.tile_pool(name="atp", bufs=at_bufs) as at_pool,
            tc.tile_pool(name="vp", bufs=2) as v_pool,
            tc.tile_pool(name="outp", bufs=2) as out_pool,
        ):
            # f32r tiles alias the fp32 bytes: the PE truncates the read to
            # fp22, cutting the 4-pass fp32 matmul to 1 pass. The ~6e-5
            # relative perturbation of v is far below the e4m3 round that
            # follows, and phase 1's earlier matmuls warm the PE for phase 2.
            xt_sb = const_pool.tile([IN_F, N], ph1_dt)
            nc.sync.dma_start(xt_sb[:], xt[:])
            wt_sb = const_pool.tile([IN_F, OUT_F], ph1_dt)
            nc.sync.dma_start(wt_sb[:], wt[:])

            at_r3 = (
                at if dr else at.rearrange("(c j p) n -> c p j n", p=P, j=jc)
            )

            # phase 1: v[m, o] = sum_i x[m, i] W[o, i], exact fp32, then
            # rounded to the matmul dtype by the DVE copy.
            v_sb = v_pool.tile([P, MC, OUT_F], a_dt, tag="v_sb")
            with tc.tile_pool(name="psv", bufs=2, space="PSUM") as psv_pool:
                for mc in range(MC):
                    ps_v = psv_pool.tile([P, OUT_F], F32)
                    nc.tensor.matmul(
                        ps_v[:],
                        xt_sb[:, mc * P : (mc + 1) * P],
                        wt_sb[:],
                        start=True,
                        stop=True,
                    )
                    nc.vector.tensor_copy(v_sb[:, mc, :], ps_v[:])

            # phase 2: y_r.T[o, n] += sum_m v[m, o] * A_r.T[m, n]
            with tc.tile_pool(name="psy", bufs=1, space="PSUM") as psy_pool:
                for _rep in range(repeat):
                    out_sb = out_pool.tile([OUT_F, N], out_dt, tag="out_sb")
                    ps_y = psy_pool.tile([OUT_F, N], F32, tag="ps_y")
                    for c in range(MC // jc):
                        at_t = at_pool.tile([P, jc, N], a_dt)
                        eng = nc.scalar if (alt and c % 2) else nc.sync
                        if variant != "pe":
                            eng.dma_start(at_t[:], at_r3[c])
                        if variant == "dma":
                            continue
                        if dr:
                            # DoubleRow: one matmul contracts a pair of
                            # 128-row blocks (256 rows), 2 elems/part/cycle.
                            npair = MC // 2
                            for jj in range(jc // 2):
                                pc = c * (jc // 2) + jj
                                for b in range(NB):
                                    nc.tensor.matmul(
                                        ps_y[:, b * BANK : (b + 1) * BANK],
                                        v_sb[:, 2 * pc : 2 * pc + 2, :],
                                        at_t[:, 2 * jj : 2 * jj + 2,
                                             b * BANK : (b + 1) * BANK],
                                        start=(pc == 0),
                                        stop=(pc == npair - 1),
                                        perf_mode=perf_mode,
                                    )
                                    if pc == npair - 1:
                                        nc.vector.tensor_copy(
                                            out_sb[:, b * BANK : (b + 1) * BANK],
                                            ps_y[:, b * BANK : (b + 1) * BANK],
                                        )
                                        oeng = nc.scalar if b % 2 else nc.sync
                                        oeng.dma_start(
                                            ytp[:, b * BANK : (b + 1) * BANK],
                                            out_sb[:, b * BANK : (b + 1) * BANK],
                                        )
                        else:
                            for j in range(jc):
                                mc = c * jc + j
                                for b in range(NB):
                                    nc.tensor.matmul(
                                        ps_y[:, b * BANK : (b + 1) * BANK],
                                        v_sb[:, mc, :],
                                        at_t[:, j, b * BANK : (b + 1) * BANK],
                                        start=(mc == 0),
                                        stop=(mc == MC - 1),
                                    )
                                    # phase 3: per-bank copy + store chase the
                                    # final matmuls
                                    if mc == MC - 1:
                                        nc.vector.tensor_copy(
                                            out_sb[:, b * BANK : (b + 1) * BANK],
                                            ps_y[:, b * BANK : (b + 1) * BANK],
                                        )
                                        nc.sync.dma_start(
                                            ytp[:, b * BANK : (b + 1) * BANK],
                                            out_sb[:, b * BANK : (b + 1) * BANK],
                                        )

    nc.compile()
    return nc


def run_with_results(inputs, repeat=1, mode=None):
    """Run the kernel; returns (full_output [4096, 64] f32, BassKernelResults)."""
    mode = mode or MODE
    adjacency = np.asarray(inputs["adjacency"], dtype=np.float32)
    x = np.asarray(inputs["x"], dtype=np.float32)
    weight = np.asarray(inputs["weight"], dtype=np.float32)
    assert adjacency.shape == (R, N, N)
    assert x.shape == (N, IN_F)
    assert weight.shape == (R, OUT_F, IN_F)

    in_maps = make_in_maps(adjacency, x, weight, mode)

    key = (repeat, mode)
    if key not in _NC_CACHE:
        _NC_CACHE[key] = _build_nc(repeat, mode)
    nc = _NC_CACHE[key]

    res = run_bass_kernel_spmd(nc, in_maps, core_ids=list(range(R)))
    return assemble_output(res.results, x, weight, mode), res


def make_in_maps(adjacency, x, weight, mode=None):
    mode = mode or MODE
    # Host-side layout prep: contraction dim must land on SBUF partitions.
    at_np = np.ascontiguousarray(adjacency.transpose(0, 2, 1))  # [R, m, n]
    if mode == "bf16":
        import ml_dtypes

        at_np = at_np.astype(ml_dtypes.bfloat16)
    elif mode == "fp8dr":
        import ml_dtypes

        # Mean-subtract before the e4m3 round: the residual is half the
        # magnitude of A, so the quantization error halves too. The exact
        # rank-1 term 0.5 * ones @ v is restored in assemble_output.
        at_np = (at_np - np.float32(0.5)).astype(ml_dtypes.float8_e4m3)
        # Pre-tile to [c, p, j, n] (row m = c*jc*128 + j*128 + p) so each
        # partition's per-tile DMA line is jc*4KB contiguous.
        jc = JC_FP8
        at_np = np.ascontiguousarray(
            at_np.reshape(R, MC // jc, jc, P, N).transpose(0, 1, 3, 2, 4)
        )
    xt_np = np.ascontiguousarray(x.T)                           # [IN_F, N]
    wt_np = np.ascontiguousarray(weight.transpose(0, 2, 1))     # [R, IN_F, OUT_F]
    return [{"at": at_np[r], "xt": xt_np, "wt": wt_np[r]} for r in range(R)]


def assemble_output(results, x=None, weight=None, mode=None):
    mode = mode or MODE
    yt = np.zeros((OUT_F, N), dtype=np.float32)
    for r in range(R):
        yt += results[r]["ytp"]
    y = np.ascontiguousarray(yt.T)
    if mode == "fp8dr":
        # Rank-1 mean correction: sum_r 0.5 * ones[N,N] @ x @ W_r.T — every
        # output row gets the same [OUT_F] vector 0.5 * (colsum x) @ W_r.T.
        colsum_x = x.astype(np.float64).sum(axis=0)             # [IN_F]
        corr = 0.5 * np.einsum(
            "i,roi->o", colsum_x, weight.astype(np.float64)
        )
        y = (y.astype(np.float64) + corr[None, :]).astype(np.float32)
    return y


def kernel(**inputs) -> np.ndarray:
    y, _ = run_with_results(inputs)
    return y


# revision 20
# speedup vs baseline: 1.1826x; 1.1826x over previous
"""Trainium2 Bass kernel for relational graph convolution:

    y = sum_r (A[r] @ x) @ W[r].T        A: [8, 4096, 4096] f32
                                         x: [4096, 64] f32, W: [8, 64, 64] f32

Strategy
--------
By associativity, y = sum_r A[r] @ v_r with v_r = x @ W[r].T, turning the
problem into one [4096, 4096] @ [4096, 64] matmul per relation. Relations are
sharded across the 8 NeuronCores (expert-style parallelism); each core returns
its partial y_r.T and the host sums and transposes.

The TensorE contracts over the partition dimension of both operands, so the
contraction index m (A's column index) must land on SBUF partitions. The host
therefore ships A[r].T (row-major) so device DMAs are plain contiguous slabs.

MODE="fp8dr" (default): the host ships e4m3(A[r].T - 0.5) — mean-subtracting
halves the quantized magnitude (and so the quantization error), and the
subtracted rank-1 term 0.5 * ones @ v_r is added back exactly on the host
(0.5 * (colsum x) @ W_r.T, identical for every output row). On device, v is
computed in exact fp32 and rounded to e4m3; phase 2 then runs
perf_mode=DoubleRow matmuls (both operands fp8): each instruction contracts
over 256 rows (2 interleaved 128-row blocks) at 2 elements/partition/cycle —
2x the bf16/f32r PE rate and 1/4 the f32 DMA traffic (16 MB/core). Measured
end-to-end rel error ~1.6e-2 (tolerance 2e-2), deterministic given the fixed
problem seed.

Per core (fp8dr):
  phase 1: v = x @ W_r.T via 32 exact-fp32 matmuls, rounded to e4m3 by the
           DVE copy into v_sb [128, 32, 64].
  phase 2: for each of 8 A-tiles (512 rows of A_r.T, 2 MB DMA): 2 DoubleRow
           pair-chunks x 8 PSUM banks of matmul(lhsT=v[128,2,64],
           rhs=A_t[128,2,512]) accumulating y_r.T [64, 4096] in fp32.
  phase 3: per-bank PSUM -> SBUF copies chase the final matmuls, then DMA
           y_r.T out; host sums partials, transposes, adds the rank-1 mean
           correction.

MODE="f32r" is the previous exact-ish variant (~1e-4 rel err, 64 MB/core,
DMA-bound); MODE="bf16" halves that traffic (~1.3e-3 rel err, 32 MB/core).
"""

import numpy as np

import concourse.tile as tile
from concourse import bacc, mybir
from concourse.bass_utils import run_bass_kernel_spmd

R, N, IN_F, OUT_F = 8, 4096, 64, 64
P = 128            # partition dim / contraction chunk
MC = N // P        # 32 contraction chunks
BANK = 512         # fp32 elems per PSUM bank
NB = N // BANK     # 8 output column blocks

F32 = mybir.dt.float32

MODE = "fp8dr"     # "fp8dr" (default), "f32r", or "bf16"
JC_FP8 = 4         # 128-row blocks per A DMA tile in fp8dr mode (even)
AT_BUFS_FP8 = 6    # A-tile prefetch depth in fp8dr mode

_NC_CACHE = {}

_A_DT = {
    "f32r": mybir.dt.float32r,
    "bf16": mybir.dt.bfloat16,
    "fp8dr": mybir.dt.float8e4,
}


def _build_nc(repeat=1, mode=None, jc=None, alt=True, at_bufs=None, variant="full"):
    """repeat>1 re-runs phase 2 (the steady-state A-streaming loop) that many
    times inside one NEFF — used only by the benchmark harness to amortize
    per-execute dispatch overhead; the graded kernel uses repeat=1.

    jc = 128-row chunks of A per DMA transfer (must be even for fp8dr);
    alt = alternate the two HWDGE rings (SP / ACT) between consecutive A-slab
    DMAs to hide the per-DMA completion gap.

    variant: "full" (the real kernel), "dma" (A-stream DMAs only — measures
    the DMA roofline; output is garbage), or "pe" (matmuls on stale SBUF, no
    A DMAs — measures the PE roofline; output is garbage). The roofline
    variants exist only for the benchmark harness."""
    mode = mode or MODE
    a_dt = _A_DT[mode]
    if jc is None:
        jc = {"f32r": 1, "bf16": 2, "fp8dr": JC_FP8}[mode]
    if at_bufs is None:
        at_bufs = {"f32r": {1: 4, 2: 3, 4: 2}[jc], "bf16": 4, "fp8dr": AT_BUFS_FP8}[
            mode
        ]
    dr = mode == "fp8dr"
    if dr:
        assert jc % 2 == 0
        perf_mode = mybir.MatmulPerfMode.DoubleRow
    if variant == "pe":
        at_bufs = MC // jc

    nc = bacc.Bacc("TRN2", target_bir_lowering=False, debug=False, num_devices=R)

    if dr:
        # Host ships A_r.T pre-tiled as [c, p, j, n] so each partition's DMA
        # line per tile is one contiguous jc*4KB run (16 KB at jc=4) — same
        # descriptor efficiency as the f32 layout's 16 KB rows.
        at = nc.dram_tensor(
            "at", [MC // jc, P, jc, N], a_dt, kind="ExternalInput"
        ).ap()
    else:
        at = nc.dram_tensor("at", [N, N], a_dt, kind="ExternalInput").ap()
    ph1_dt = mybir.dt.float32r if dr else F32
    # bf16 partials halve phase-3 traffic; the host accumulates them in fp32.
    # The extra ~7e-4 relative error is far below the e4m3 quantization term.
    out_dt = mybir.dt.bfloat16 if dr else F32
    xt = nc.dram_tensor("xt", [IN_F, N], ph1_dt, kind="ExternalInput").ap()
    wt = nc.dram_tensor("wt", [IN_F, OUT_F], ph1_dt, kind="ExternalInput").ap()
    ytp = nc.dram_tensor("ytp", [OUT_F, N], out_dt, kind="ExternalOutput").ap()

    with tile.TileContext(nc) as tc:
        with (
            tc.tile_pool(name="const", bufs=1) as const_pool,
            tc.tile_pool(name="atp", bufs=at_bufs) as at_pool,
            tc.tile_pool(name="vp", bufs=2) as v_pool,
            tc.tile_pool(name="outp", bufs=2) as out_pool,
        ):
            # f32r tiles alias the fp32 bytes: the PE truncates the read to
            # fp22, cutting the 4-pass fp32 matmul to 1 pass. The ~6e-5
            # relative perturbation of v is far below the e4m3 round that
            # follows, and phase 1's earlier matmuls warm the PE for phase 2.
            xt_sb = const_pool.tile([IN_F, N], ph1_dt)
            nc.sync.dma_start(xt_sb[:], xt[:])
            wt_sb = const_pool.tile([IN_F, OUT_F], ph1_dt)
            nc.sync.dma_start(wt_sb[:], wt[:])

            at_r3 = (
                at if dr else at.rearrange("(c j p) n -> c p j n", p=P, j=jc)
            )

            # phase 1: v[m, o] = sum_i x[m, i] W[o, i], exact fp32, then
            # rounded to the matmul dtype by the DVE copy.
            v_sb = v_pool.tile([P, MC, OUT_F], a_dt, tag="v_sb")
            with tc.tile_pool(name="psv", bufs=2, space="PSUM") as psv_pool:
                for mc in range(MC):
                    ps_v = psv_pool.tile([P, OUT_F], F32)
                    nc.tensor.matmul(
                        ps_v[:],
                        xt_sb[:, mc * P : (mc + 1) * P],
                        wt_sb[:],
                        start=True,
                        stop=True,
                    )
                    nc.vector.tensor_copy(v_sb[:, mc, :], ps_v[:])

            # phase 2: y_r.T[o, n] += sum_m v[m, o] * A_r.T[m, n]
            pe_tiles = None
            if variant == "pe":
                # Preload every A tile once; the repeat loop then reruns the
                # matmul stream with zero DMA traffic (PE roofline probe).
                pe_tiles = [at_pool.tile([P, jc, N], a_dt) for _ in range(MC // jc)]
                for c, t in enumerate(pe_tiles):
                    eng = nc.scalar if (alt and c % 2) else nc.sync
                    eng.dma_start(t[:], at_r3[c])
            with tc.tile_pool(name="psy", bufs=1, space="PSUM") as psy_pool:
                for _rep in range(repeat):
                    out_sb = out_pool.tile([OUT_F, N], out_dt, tag="out_sb")
                    ps_y = psy_pool.tile([OUT_F, N], F32, tag="ps_y")
                    for c in range(MC // jc):
                        if variant == "pe":
                            at_t = pe_tiles[c]
                        else:
                            at_t = at_pool.tile([P, jc, N], a_dt)
                            eng = nc.scalar if (alt and c % 2) else nc.sync
                            eng.dma_start(at_t[:], at_r3[c])
                        if variant == "dma":
                            continue
                        if dr:
                            # DoubleRow: one matmul contracts a pair of
                            # 128-row blocks (256 rows), 2 elems/part/cycle.
                            npair = MC // 2
                            for jj in range(jc // 2):
                                pc = c * (jc // 2) + jj
                                for b in range(NB):
                                    nc.tensor.matmul(
                                        ps_y[:, b * BANK : (b + 1) * BANK],
                                        v_sb[:, 2 * pc : 2 * pc + 2, :],
                                        at_t[:, 2 * jj : 2 * jj + 2,
                                             b * BANK : (b + 1) * BANK],
                                        start=(pc == 0),
                                        stop=(pc == npair - 1),
                                        perf_mode=perf_mode,
                                    )
                                    if pc == npair - 1:
                                        nc.vector.tensor_copy(
                                            out_sb[:, b * BANK : (b + 1) * BANK],
                                            ps_y[:, b * BANK : (b + 1) * BANK],
                                        )
                                        oeng = nc.scalar if b % 2 else nc.sync
                                        oeng.dma_start(
                                            ytp[:, b * BANK : (b + 1) * BANK],
                                            out_sb[:, b * BANK : (b + 1) * BANK],
                                        )
                        else:
                            for j in range(jc):
                                mc = c * jc + j
                                for b in range(NB):
                                    nc.tensor.matmul(
                                        ps_y[:, b * BANK : (b + 1) * BANK],
                                        v_sb[:, mc, :],
                                        at_t[:, j, b * BANK : (b + 1) * BANK],
                                        start=(mc == 0),
                                        stop=(mc == MC - 1),
                                    )
                                    # phase 3: per-bank copy + store chase the
                                    # final matmuls
                                    if mc == MC - 1:
                                        nc.vector.tensor_copy(
                                            out_sb[:, b * BANK : (b + 1) * BANK],
                                            ps_y[:, b * BANK : (b + 1) * BANK],
                                        )
                                        nc.sync.dma_start(
                                            ytp[:, b * BANK : (b + 1) * BANK],
                                            out_sb[:, b * BANK : (b + 1) * BANK],
                                        )

    nc.compile()
    return nc


def run_with_results(inputs, repeat=1, mode=None):
    """Run the kernel; returns (full_output [4096, 64] f32, BassKernelResults)."""
    mode = mode or MODE
    adjacency = np.asarray(inputs["adjacency"], dtype=np.float32)
    x = np.asarray(inputs["x"], dtype=np.float32)
    weight = np.asarray(inputs["weight"], dtype=np.float32)
    assert adjacency.shape == (R, N, N)
    assert x.shape == (N, IN_F)
    assert weight.shape == (R, OUT_F, IN_F)

    in_maps = make_in_maps(adjacency, x, weight, mode)

    key = (repeat, mode)
    if key not in _NC_CACHE:
        _NC_CACHE[key] = _build_nc(repeat, mode)
    nc = _NC_CACHE[key]

    res = run_bass_kernel_spmd(nc, in_maps, core_ids=list(range(R)))
    return assemble_output(res.results, x, weight, mode), res


def make_in_maps(adjacency, x, weight, mode=None):
    mode = mode or MODE
    # Host-side layout prep: contraction dim must land on SBUF partitions.
    at_np = np.ascontiguousarray(adjacency.transpose(0, 2, 1))  # [R, m, n]
    if mode == "bf16":
        import ml_dtypes

        at_np = at_np.astype(ml_dtypes.bfloat16)
    elif mode == "fp8dr":
        import ml_dtypes

        # Mean-subtract before the e4m3 round: the residual is half the
        # magnitude of A, so the quantization error halves too. The exact
        # rank-1 term 0.5 * ones @ v is restored in assemble_output.
        at_np = (at_np - np.float32(0.5)).astype(ml_dtypes.float8_e4m3)
        # Pre-tile to [c, p, j, n] (row m = c*jc*128 + j*128 + p) so each
        # partition's per-tile DMA line is jc*4KB contiguous.
        jc = JC_FP8
        at_np = np.ascontiguousarray(
            at_np.reshape(R, MC // jc, jc, P, N).transpose(0, 1, 3, 2, 4)
        )
    xt_np = np.ascontiguousarray(x.T)                           # [IN_F, N]
    wt_np = np.ascontiguousarray(weight.transpose(0, 2, 1))     # [R, IN_F, OUT_F]
    return [{"at": at_np[r], "xt": xt_np, "wt": wt_np[r]} for r in range(R)]


def assemble_output(results, x=None, weight=None, mode=None):
    mode = mode or MODE
    yt = np.zeros((OUT_F, N), dtype=np.float32)
    for r in range(R):
        yt += results[r]["ytp"]
    y = np.ascontiguousarray(yt.T)
    if mode == "fp8dr":
        # Rank-1 mean correction: sum_r 0.5 * ones[N,N] @ x @ W_r.T — every
        # output row gets the same [OUT_F] vector 0.5 * (colsum x) @ W_r.T.
        colsum_x = x.astype(np.float64).sum(axis=0)             # [IN_F]
        corr = 0.5 * np.einsum(
            "i,roi->o", colsum_x, weight.astype(np.float64)
        )
        y = (y.astype(np.float64) + corr[None, :]).astype(np.float32)
    return y


def kernel(**inputs) -> np.ndarray:
    y, _ = run_with_results(inputs)
    return y
